# revision 7
# baseline (speedup 1.0000x reference)
"""Trainium2 Bass kernel for nn_Agent_74844100100112 (energy-based policy sampler).

Strategy:
 - Host (inside kernel()): reproduce the jax PRNG noise exactly (fixed key 42),
   fold stepsize/noise_scale/We3 into weights & offsets, shard batch over 8 cores
   (512 batch rows x 11 chains = 5632 rows/core).
 - Device: 10 Langevin steps; per step, per chain-chunk of 512 rows:
   fwd 72->256->256 (bf16 matmuls, tanh on ACT), backward via
     NSQ2=-H2^2; DH1 = We2R @ NSQ2 + crow (c-matmul trick);
     G1 = (1+NSQ1)*DH1; GA = -(s/2)*We1a @ G1 (scale folded into weights);
   action update in fp32 on packed "quad" tiles [128,512] (chunk j at
   partitions 32j..32j+8):  A = clip(A + clip(GA,+-s/2) - OFF, -1, 1).
   Then a final energy forward pass (fp32r) and a 4-layer critic (fp32).
 - Host: gather energies, logsumexp over the 10 sampler chains, log_prob,
   entropy, assemble outputs.
"""

import numpy as np
import ml_dtypes

import concourse.bass as bass
from concourse import bacc
import concourse.mybir as mybir
import concourse.tile as tile
from concourse import bass_utils

AFT = mybir.ActivationFunctionType
ALU = mybir.AluOpType
F32 = mybir.dt.float32
F32R = mybir.dt.float32r
BF16 = mybir.dt.bfloat16

C, B, ACTD, OBS, HID = 11, 4096, 8, 64, 256
NSTEP = 10
NCORE = 8
BC = B // NCORE          # 512 batch rows per core
NCHUNK = C               # chunk == chain (512 rows each)
NQ = 3                   # quads of 4 chunks

STEP_INIT = np.float32(0.1)
STEP_FINAL = np.float32(0.1 * 0.1)


def _stepsizes():
    out = []
    for t in range(NSTEP):
        s = (STEP_INIT - STEP_FINAL) * (np.float32(1.0) - np.float32(t) / np.float32(9.0)) ** 2 + STEP_FINAL
        out.append(np.float32(s))
    return out


# ---------------------------------------------------------------- device build

_CACHE = {}


def _build():
    if "nc" in _CACHE:
        return _CACHE["nc"]

    nc = bacc.Bacc(trn_type="TRN2")
    steps = _stepsizes()

    # ---- DRAM I/O
    d_st_f = nc.dram_tensor("st_f", [OBS, BC], F32, kind="ExternalInput")
    d_st_b = nc.dram_tensor("st_b", [OBS, BC], BF16, kind="ExternalInput")
    d_aq0 = nc.dram_tensor("aq0", [NQ, 128, BC], F32, kind="ExternalInput")
    d_offq = nc.dram_tensor("offq", [NSTEP, NQ, 128, BC], F32, kind="ExternalInput")

    d_we1s_b = nc.dram_tensor("we1s_b", [OBS, HID], BF16, kind="ExternalInput")
    d_we1a_b = nc.dram_tensor("we1a_b", [128, HID], BF16, kind="ExternalInput")
    d_we2_b = nc.dram_tensor("we2_b", [HID, HID], BF16, kind="ExternalInput")
    d_we2rT_b = nc.dram_tensor("we2rT_b", [HID, HID], BF16, kind="ExternalInput")
    d_crow_b = nc.dram_tensor("crow_b", [1, HID], BF16, kind="ExternalInput")
    d_waTs_b = nc.dram_tensor("waTs_b", [128, NSTEP * 2 * ACTD], BF16, kind="ExternalInput")

    d_we1s_f = nc.dram_tensor("we1s_f", [OBS, HID], F32R, kind="ExternalInput")
    d_we1a_f = nc.dram_tensor("we1a_f", [128, HID], F32R, kind="ExternalInput")
    d_we2_f = nc.dram_tensor("we2_f", [HID, HID], F32R, kind="ExternalInput")
    d_we3_f = nc.dram_tensor("we3_f", [128, 2], F32R, kind="ExternalInput")
    d_wc1_f = nc.dram_tensor("wc1_f", [OBS, HID], F32, kind="ExternalInput")
    d_wc2_f = nc.dram_tensor("wc2_f", [HID, HID], F32, kind="ExternalInput")
    d_wc3_f = nc.dram_tensor("wc3_f", [HID, HID], F32, kind="ExternalInput")
    d_wc4_f = nc.dram_tensor("wc4_f", [128, 2], F32, kind="ExternalInput")
    d_be1 = nc.dram_tensor("be1", [128, 2], F32, kind="ExternalInput")
    d_be2 = nc.dram_tensor("be2", [128, 2], F32, kind="ExternalInput")
    d_bc1 = nc.dram_tensor("bc1", [128, 2], F32, kind="ExternalInput")
    d_bc2 = nc.dram_tensor("bc2", [128, 2], F32, kind="ExternalInput")
    d_bc3 = nc.dram_tensor("bc3", [128, 2], F32, kind="ExternalInput")

    d_a_out = nc.dram_tensor("a_out", [ACTD, BC], F32, kind="ExternalOutput")
    d_e_out = nc.dram_tensor("e_out", [1, C * BC], F32, kind="ExternalOutput")
    d_v_out = nc.dram_tensor("v_out", [1, BC], F32, kind="ExternalOutput")

    with tile.TileContext(nc) as tc:
        with (
            tc.tile_pool(name="const", bufs=1) as cp,
            tc.tile_pool(name="work", bufs=2) as sb,
            tc.tile_pool(name="psum", bufs=1, space="PSUM") as pp,
        ):
            # ---- persistent tiles
            st_f = cp.tile([OBS, BC], F32)
            st_b = cp.tile([OBS, BC], BF16)
            we1s_b = cp.tile([OBS, HID], BF16)
            we1a_b = cp.tile([128, HID], BF16)
            we2_b = [cp.tile([128, HID], BF16, tag=f"we2b{k}", name=f"we2b{k}") for k in range(2)]
            we2rT_b = [cp.tile([128, HID], BF16, tag=f"we2rTb{k}", name=f"we2rTb{k}") for k in range(2)]
            crow_b = cp.tile([1, HID], BF16)
            waTs_b = cp.tile([128, NSTEP * 2 * ACTD], BF16)
            we1s_f = cp.tile([OBS, HID], F32R)
            we1a_f = cp.tile([128, HID], F32R)
            we2_f = [cp.tile([128, HID], F32R, tag=f"we2f{k}", name=f"we2f{k}") for k in range(2)]
            we3_f = cp.tile([128, 2], F32R)
            wc1_f = cp.tile([OBS, HID], F32)
            wc2_f = [cp.tile([128, HID], F32, tag=f"wc2f{k}", name=f"wc2f{k}") for k in range(2)]
            wc3_f = [cp.tile([128, HID], F32, tag=f"wc3f{k}", name=f"wc3f{k}") for k in range(2)]
            wc4_f = cp.tile([128, 2], F32)
            be1 = cp.tile([128, 2], F32)
            be2 = cp.tile([128, 2], F32)
            bc1 = cp.tile([128, 2], F32)
            bc2 = cp.tile([128, 2], F32)
            bc3 = cp.tile([128, 2], F32)
            ones_b = cp.tile([1, BC], BF16)
            aq = [cp.tile([128, BC], F32, tag=f"aq{q}", name=f"aq{q}") for q in range(NQ)]
            ab = [cp.tile([128, BC], BF16, tag=f"ab{q}", name=f"ab{q}") for q in range(NQ)]
            uq = [cp.tile([128, BC], F32, tag=f"uq{q}", name=f"uq{q}") for q in range(NQ)]
            st_r = cp.tile([OBS, BC], F32R)
            af = [cp.tile([128, BC], F32R, tag=f"af{q}", name=f"af{q}") for q in range(NQ)]
            e_sb = cp.tile([1, C * BC], F32)
            v_sb = cp.tile([1, BC], F32)

            # ---- loads
            dma = nc.sync.dma_start
            dma(st_f[:], d_st_f[:])
            dma(st_b[:], d_st_b[:])
            dma(we1s_b[:], d_we1s_b[:])
            dma(we1a_b[:], d_we1a_b[:])
            for k in range(2):
                dma(we2_b[k][:], d_we2_b[k * 128:(k + 1) * 128, :])
                dma(we2rT_b[k][:], d_we2rT_b[k * 128:(k + 1) * 128, :])
                dma(we2_f[k][:], d_we2_f[k * 128:(k + 1) * 128, :])
                dma(wc2_f[k][:], d_wc2_f[k * 128:(k + 1) * 128, :])
                dma(wc3_f[k][:], d_wc3_f[k * 128:(k + 1) * 128, :])
            dma(crow_b[:], d_crow_b[:])
            dma(waTs_b[:], d_waTs_b[:])
            dma(we1s_f[:], d_we1s_f[:])
            dma(we1a_f[:], d_we1a_f[:])
            dma(we3_f[:], d_we3_f[:])
            dma(wc1_f[:], d_wc1_f[:])
            dma(wc4_f[:], d_wc4_f[:])
            dma(be1[:], d_be1[:])
            dma(be2[:], d_be2[:])
            dma(bc1[:], d_bc1[:])
            dma(bc2[:], d_bc2[:])
            dma(bc3[:], d_bc3[:])
            for q in range(NQ):
                dma(aq[q][:], d_aq0[q])
                nc.vector.memset(uq[q][:], 0.0)
                nc.vector.tensor_copy(ab[q][:], aq[q][:])
            nc.gpsimd.memset(ones_b[:], 1.0)

            mm = nc.tensor.matmul
            act = nc.scalar.activation
            stt = nc.vector.scalar_tensor_tensor
            ts = nc.vector.tensor_scalar

            def quad_update(t, q, offt):
                s2 = float(steps[t]) / 2.0
                x = sb.tile([128, BC], F32, tag="x", bufs=2)
                stt(x[:], offt[q][:], -1.0, uq[q][:], op0=ALU.mult, op1=ALU.add)
                nc.vector.tensor_add(aq[q][:], aq[q][:], x[:])
                ts(aq[q][:], aq[q][:], 1.0, -1.0, op0=ALU.min, op1=ALU.max)
                nc.vector.tensor_copy(ab[q][:], aq[q][:])

            # ================= MCMC loop =================
            for t in range(NSTEP):
                s2 = float(steps[t]) / 2.0
                offt = []
                for q in range(NQ):
                    o = sb.tile([128, BC], F32, tag="off", bufs=3)
                    dma(o[:], d_offq[t, q])
                    offt.append(o)
                for c in range(NCHUNK):
                    q, j = divmod(c, 4)
                    a_sl = ab[q][32 * j:32 * j + 8, :]
                    pre1 = [pp.tile([128, BC], F32, tag=f"pre1_{h}", bufs=1, name=f"pre1_{h}") for h in range(2)]
                    for h in range(2):
                        hs = slice(h * 128, (h + 1) * 128)
                        mm(pre1[h][:], we1s_b[:, hs], st_b[:], start=True, stop=False)
                        mm(pre1[h][:], we1a_b[32 * j:32 * j + 8, hs], a_sl, start=False, stop=True, tile_position=(32 * j, 0))
                    h1 = [sb.tile([128, BC], BF16, tag=f"h1_{h}", bufs=2, name=f"h1_{h}") for h in range(2)]
                    for h in range(2):
                        act(h1[h][:], pre1[h][:], AFT.Tanh, bias=be1[:, h:h + 1])
                    pre2 = [pp.tile([128, BC], F32, tag=f"pre2_{h}", bufs=1, name=f"pre2_{h}") for h in range(2)]
                    for h in range(2):
                        hs = slice(h * 128, (h + 1) * 128)
                        mm(pre2[h][:], we2_b[0][:, hs], h1[0][:], start=True, stop=False)
                        mm(pre2[h][:], we2_b[1][:, hs], h1[1][:], start=False, stop=True)
                    h2 = [sb.tile([128, BC], BF16, tag=f"h2_{h}", bufs=2, name=f"h2_{h}") for h in range(2)]
                    for h in range(2):
                        act(h2[h][:], pre2[h][:], AFT.Tanh, bias=be2[:, h:h + 1])
                    nsq2 = [sb.tile([128, BC], BF16, tag=f"nsq2_{h}", bufs=2, name=f"nsq2_{h}") for h in range(2)]
                    nsq1 = [sb.tile([128, BC], BF16, tag=f"nsq1_{h}", bufs=2, name=f"nsq1_{h}") for h in range(2)]
                    for h in range(2):
                        stt(nsq2[h][:], h2[h][:], -1.0, h2[h][:], op0=ALU.mult, op1=ALU.mult)
                        stt(nsq1[h][:], h1[h][:], -1.0, h1[h][:], op0=ALU.mult, op1=ALU.mult)
                    dh1 = [pp.tile([128, BC], F32, tag=f"dh1_{h}", bufs=1, name=f"dh1_{h}") for h in range(2)]
                    for h in range(2):
                        hs = slice(h * 128, (h + 1) * 128)
                        mm(dh1[h][:], we2rT_b[0][:, hs], nsq2[0][:], start=True, stop=False)
                        mm(dh1[h][:], we2rT_b[1][:, hs], nsq2[1][:], start=False, stop=False)
                        mm(dh1[h][:], crow_b[0:1, hs], ones_b[0:1, :], start=False, stop=True)
                    g1 = [sb.tile([128, BC], BF16, tag=f"g1_{h}", bufs=2, name=f"g1_{h}") for h in range(2)]
                    for h in range(2):
                        stt(g1[h][:], nsq1[h][:], 1.0, dh1[h][:], op0=ALU.add, op1=ALU.mult)
                    ga = pp.tile([ACTD, BC], F32, tag="ga", bufs=2)
                    for k in range(2):
                        ws = slice((t * 2 + k) * ACTD, (t * 2 + k + 1) * ACTD)
                        mm(ga[:], waTs_b[:, ws], g1[k][:], start=(k == 0), stop=(k == 1))
                    ts(uq[q][32 * j:32 * j + 8, :], ga[:], s2, -s2, op0=ALU.min, op1=ALU.max)
                    if j == 3 or c == NCHUNK - 1:
                        quad_update(t, q, offt)

            # ================= final energy pass (fp32r) =================
            nc.vector.tensor_copy(st_r[:], st_f[:])
            for q in range(NQ):
                nc.vector.tensor_copy(af[q][:], aq[q][:])
            for c in range(NCHUNK):
                q, j = divmod(c, 4)
                a_sl = af[q][32 * j:32 * j + 8, :]
                pre1 = [pp.tile([128, BC], F32, tag=f"pre1_{h}", bufs=1, name=f"pre1_{h}") for h in range(2)]
                for h in range(2):
                    hs = slice(h * 128, (h + 1) * 128)
                    mm(pre1[h][:], we1s_f[:, hs], st_r[:], start=True, stop=False)
                    mm(pre1[h][:], we1a_f[32 * j:32 * j + 8, hs], a_sl, start=False, stop=True, tile_position=(32 * j, 0))
                h1f = [sb.tile([128, BC], F32R, tag=f"h1f_{h}", bufs=2, name=f"h1f_{h}") for h in range(2)]
                for h in range(2):
                    act(h1f[h][:], pre1[h][:], AFT.Tanh, bias=be1[:, h:h + 1])
                pre2 = [pp.tile([128, BC], F32, tag=f"pre2_{h}", bufs=1, name=f"pre2_{h}") for h in range(2)]
                for h in range(2):
                    hs = slice(h * 128, (h + 1) * 128)
                    mm(pre2[h][:], we2_f[0][:, hs], h1f[0][:], start=True, stop=False)
                    mm(pre2[h][:], we2_f[1][:, hs], h1f[1][:], start=False, stop=True)
                h2f = [sb.tile([128, BC], F32R, tag=f"h2f_{h}", bufs=2, name=f"h2f_{h}") for h in range(2)]
                for h in range(2):
                    act(h2f[h][:], pre2[h][:], AFT.Tanh, bias=be2[:, h:h + 1])
                ev = pp.tile([1, BC], F32, tag="ga", bufs=2)
                for k in range(2):
                    mm(ev[:], we3_f[:, k:k + 1], h2f[k][:], start=(k == 0), stop=(k == 1))
                nc.scalar.copy(e_sb[0:1, c * BC:(c + 1) * BC], ev[:])

            # ================= critic (fp32) =================
            cpre = [pp.tile([128, BC], F32, tag=f"pre1_{h}", bufs=1, name=f"pre1_{h}") for h in range(2)]
            for h in range(2):
                hs = slice(h * 128, (h + 1) * 128)
                mm(cpre[h][:], wc1_f[:, hs], st_f[:], start=True, stop=True)
            t1 = [sb.tile([128, BC], F32, tag=f"ct1_{h}", bufs=1, name=f"ct1_{h}") for h in range(2)]
            for h in range(2):
                act(t1[h][:], cpre[h][:], AFT.Tanh, bias=bc1[:, h:h + 1])
            cpre2 = [pp.tile([128, BC], F32, tag=f"pre2_{h}", bufs=1, name=f"pre2_{h}") for h in range(2)]
            for h in range(2):
                hs = slice(h * 128, (h + 1) * 128)
                mm(cpre2[h][:], wc2_f[0][:, hs], t1[0][:], start=True, stop=False)
                mm(cpre2[h][:], wc2_f[1][:, hs], t1[1][:], start=False, stop=True)
            t2 = [sb.tile([128, BC], F32, tag=f"ct2_{h}", bufs=1, name=f"ct2_{h}") for h in range(2)]
            for h in range(2):
                act(t2[h][:], cpre2[h][:], AFT.Tanh, bias=bc2[:, h:h + 1])
            cpre3 = [pp.tile([128, BC], F32, tag=f"pre1_{h}", bufs=1, name=f"pre1_{h}") for h in range(2)]
            for h in range(2):
                hs = slice(h * 128, (h + 1) * 128)
                mm(cpre3[h][:], wc3_f[0][:, hs], t2[0][:], start=True, stop=False)
                mm(cpre3[h][:], wc3_f[1][:, hs], t2[1][:], start=False, stop=True)
            t3 = [sb.tile([128, BC], F32, tag=f"ct3_{h}", bufs=1, name=f"ct3_{h}") for h in range(2)]
            for h in range(2):
                act(t3[h][:], cpre3[h][:], AFT.Tanh, bias=bc3[:, h:h + 1])
            vv = pp.tile([1, BC], F32, tag="ga", bufs=2)
            for k in range(2):
                mm(vv[:], wc4_f[:, k:k + 1], t3[k][:], start=(k == 0), stop=(k == 1))
            nc.scalar.copy(v_sb[:], vv[:])

            # ================= outputs =================
            dma(d_a_out[:], aq[0][0:8, :])
            dma(d_e_out[:], e_sb[:])
            dma(d_v_out[:], v_sb[:])

    nc.finalize()
    _CACHE["nc"] = nc
    return nc


# ---------------------------------------------------------------- host side

def _host_noise(temperature):
    import jax
    import jax.numpy as jnp
    cpu = jax.devices("cpu")[0]
    with jax.default_device(cpu):
        T = np.float32(np.exp(np.float32(temperature[0])))
        key = jax.random.key(42)
        k_init, k_loop = jax.random.split(key)
        a0 = np.asarray(jnp.tanh(jax.random.normal(k_init, (C, B, ACTD))))
        noises = []
        k = k_loop
        for _ in range(NSTEP):
            k, kn = jax.random.split(k)
            noises.append(np.asarray(jax.random.normal(kn, (C, B, ACTD))))
    steps = _stepsizes()
    off = [np.float32(steps[t] * T) * noises[t] for t in range(NSTEP)]
    return a0, off, steps, T


def _to_quads(arr):
    """arr [C, BCrows, ACTD] -> [NQ, 128, BC] (chunk j of quad at partitions 32j..32j+8)."""
    out = np.zeros((NQ, 128, BC), np.float32)
    for i in range(C):
        q, j = divmod(i, 4)
        out[q, 32 * j:32 * j + 8, :] = arr[i].T
    return out


def _prep_in_maps(inputs):
    f32 = lambda x: np.ascontiguousarray(np.asarray(x, np.float32))
    b16 = lambda x: np.ascontiguousarray(np.asarray(x, np.float32).astype(ml_dtypes.bfloat16))
    states = f32(inputs["states"])
    We1 = f32(inputs["We1"]); be1 = f32(inputs["be1"])
    We2 = f32(inputs["We2"]); be2 = f32(inputs["be2"])
    We3 = f32(inputs["We3"]); be3 = f32(inputs["be3"])
    Wc1 = f32(inputs["Wc1"]); bc1 = f32(inputs["bc1"])
    Wc2 = f32(inputs["Wc2"]); bc2 = f32(inputs["bc2"])
    Wc3 = f32(inputs["Wc3"]); bc3 = f32(inputs["bc3"])
    Wc4 = f32(inputs["Wc4"]); bc4 = f32(inputs["bc4"])
    temperature = f32(inputs["temperature"])

    a0, off, steps, T = _host_noise(temperature)

    we3v = We3[:, 0]
    we2rT = we3v[:, None] * We2.T
    crow = (We2 @ we3v)[None, :]                      # [1, 256]
    waTs = np.zeros((128, NSTEP * 2 * ACTD), np.float32)
    for t in range(NSTEP):
        w = np.float32(-(steps[t] / np.float32(2.0))) * We1[OBS:].T    # [256, 8]
        for k in range(2):
            waTs[:, (t * 2 + k) * ACTD:(t * 2 + k + 1) * ACTD] = w[k * 128:(k + 1) * 128]

    pack2 = lambda v: np.stack([v[:128], v[128:]], axis=1)  # [256] -> [128, 2]
    we1a4 = np.zeros((128, HID), np.float32)
    for j in range(4):
        we1a4[32 * j:32 * j + ACTD] = We1[OBS:]
    shared = {
        "we1s_b": b16(We1[:OBS]), "we1a_b": b16(we1a4),
        "we2_b": b16(We2), "we2rT_b": b16(we2rT), "crow_b": b16(crow),
        "waTs_b": b16(waTs),
        "we1s_f": We1[:OBS].copy(), "we1a_f": we1a4, "we2_f": We2,
        "we3_f": pack2(We3[:, 0]), "wc1_f": Wc1, "wc2_f": Wc2, "wc3_f": Wc3,
        "wc4_f": pack2(Wc4[:, 0]),
        "be1": pack2(be1), "be2": pack2(be2),
        "bc1": pack2(bc1), "bc2": pack2(bc2), "bc3": pack2(bc3),
    }

    in_maps = []
    for cc in range(NCORE):
        bs = slice(cc * BC, (cc + 1) * BC)
        st = np.ascontiguousarray(states[bs].T)           # [64, 512]
        aq0 = _to_quads(a0[:, bs, :])
        offq = np.stack([_to_quads(off[t][:, bs, :]) for t in range(NSTEP)])
        m = dict(shared)
        m.update({"st_f": st, "st_b": b16(st), "aq0": aq0, "offq": offq})
        in_maps.append(m)

    meta = {"T": T, "be3": np.float32(be3[0]), "bc4": np.float32(bc4[0])}
    return in_maps, meta


def _postprocess(results, meta):
    T = meta["T"]
    actions = np.zeros((B, ACTD), np.float32)
    E = np.zeros((C, B), np.float32)
    V = np.zeros((B, 1), np.float32)
    for cc, r in enumerate(results):
        bs = slice(cc * BC, (cc + 1) * BC)
        actions[bs] = r["a_out"].T
        E[:, bs] = r["e_out"].reshape(C, BC)
        V[bs, 0] = r["v_out"][0]
    E += meta["be3"]
    V += meta["bc4"]
    energy = E[0]
    neg = -E[1:] / T
    m = neg.max(axis=0)
    lse = m + np.log(np.exp(neg - m).sum(axis=0))
    log_prob = (-energy / T - lse).astype(np.float32)
    entropy = np.float32(T * log_prob.mean(dtype=np.float64))
    return actions, log_prob, entropy, V


def _run(inputs, **spmd_kwargs):
    nc = _build()
    in_maps, meta = _prep_in_maps(inputs)
    res = bass_utils.run_bass_kernel_spmd(nc, in_maps, core_ids=list(range(NCORE)), **spmd_kwargs)
    return _postprocess(res.results, meta), res


def kernel(**inputs):
    out, _ = _run(inputs)
    return out


# revision 9
# speedup vs baseline: 1.0362x; 1.0362x over previous
"""Trainium2 Bass kernel for nn_Agent_74844100100112 (energy-based policy sampler).

Strategy:
 - Host (inside kernel()): reproduce the jax PRNG noise exactly (fixed key 42),
   fold stepsize/noise_scale/We3 into weights & offsets, shard batch over 8 cores
   (512 batch rows x 11 chains = 5632 rows/core).
 - Device: 10 Langevin steps; per step, per chain-chunk of 512 rows (bf16):
     fwd: pre1 = [s;1] @ [We1s;be1] + a @ We1a (K=65 + K=8 matmuls), tanh;
          pre2 = h1 @ We2 + be2 (bias via ACT), tanh
     bwd: sq = h^2 (tt), d = 1-sq (ts), DH1 = We2R @ d2 (matmul, We3 folded),
          g1 = d1*dh1 (tt), GA = -(s/2)*We1a @ g1 (stepsize folded in weights)
     GA col-tiled: 4 chunks -> one PSUM bank; one clip per quad:
          u = clip(GA, +-s/2);  A = clip(A + u - OFF, -1, 1)  (fp32 quads)
 - Final energy pass in fp32r, critic in fp32.
 - Host: logsumexp over 10 sampler chains, log_prob, entropy, assemble.
"""

import numpy as np
import ml_dtypes

import concourse.bass as bass
from concourse import bacc
import concourse.mybir as mybir
import concourse.tile as tile
from concourse import bass_utils

AFT = mybir.ActivationFunctionType
ALU = mybir.AluOpType
F32 = mybir.dt.float32
F32R = mybir.dt.float32r
BF16 = mybir.dt.bfloat16

C, B, ACTD, OBS, HID = 11, 4096, 8, 64, 256
NSTEP = 10
NCORE = 8
BC = B // NCORE          # 512 batch rows per core
NCHUNK = C               # chunk == chain (512 rows each)
NQ = 3                   # quads of 4 chunks

STEP_INIT = np.float32(0.1)
STEP_FINAL = np.float32(0.1 * 0.1)


def _stepsizes():
    out = []
    for t in range(NSTEP):
        s = (STEP_INIT - STEP_FINAL) * (np.float32(1.0) - np.float32(t) / np.float32(9.0)) ** 2 + STEP_FINAL
        out.append(np.float32(s))
    return out


# ---------------------------------------------------------------- device build

_CACHE = {}


def _build():
    if "nc" in _CACHE:
        return _CACHE["nc"]

    nc = bacc.Bacc(trn_type="TRN2")
    steps = _stepsizes()

    # ---- DRAM I/O (st has a trailing ones-row: K=65; we1s has be1 as row 64)
    d_st_b = nc.dram_tensor("st_b", [OBS + 1, BC], BF16, kind="ExternalInput")
    d_st_r = nc.dram_tensor("st_r", [OBS + 1, BC], F32R, kind="ExternalInput")
    d_st_f = nc.dram_tensor("st_f", [OBS + 1, BC], F32, kind="ExternalInput")
    d_aq0 = nc.dram_tensor("aq0", [NQ, 128, BC], F32, kind="ExternalInput")
    d_offq = nc.dram_tensor("offq", [NSTEP, NQ, 128, BC], F32, kind="ExternalInput")

    d_we1s_b = nc.dram_tensor("we1s_b", [OBS + 1, HID], BF16, kind="ExternalInput")
    d_we1a_b = nc.dram_tensor("we1a_b", [128, HID], BF16, kind="ExternalInput")
    d_we2_b = nc.dram_tensor("we2_b", [HID, HID], BF16, kind="ExternalInput")
    d_we2rT_b = nc.dram_tensor("we2rT_b", [HID, HID], BF16, kind="ExternalInput")
    d_waTs_b = nc.dram_tensor("waTs_b", [128, NSTEP * 2 * ACTD], BF16, kind="ExternalInput")

    d_we1s_f = nc.dram_tensor("we1s_f", [OBS + 1, HID], F32R, kind="ExternalInput")
    d_we1a_f = nc.dram_tensor("we1a_f", [128, HID], F32R, kind="ExternalInput")
    d_we2_f = nc.dram_tensor("we2_f", [HID, HID], F32R, kind="ExternalInput")
    d_we3_f = nc.dram_tensor("we3_f", [128, 2], F32R, kind="ExternalInput")
    d_wc1_f = nc.dram_tensor("wc1_f", [OBS + 1, HID], F32, kind="ExternalInput")
    d_wc2_f = nc.dram_tensor("wc2_f", [HID, HID], F32, kind="ExternalInput")
    d_wc3_f = nc.dram_tensor("wc3_f", [HID, HID], F32, kind="ExternalInput")
    d_wc4_f = nc.dram_tensor("wc4_f", [128, 2], F32, kind="ExternalInput")
    d_be2 = nc.dram_tensor("be2", [128, 2], F32, kind="ExternalInput")
    d_bc2 = nc.dram_tensor("bc2", [128, 2], F32, kind="ExternalInput")
    d_bc3 = nc.dram_tensor("bc3", [128, 2], F32, kind="ExternalInput")

    d_a_out = nc.dram_tensor("a_out", [ACTD, BC], F32, kind="ExternalOutput")
    d_e_out = nc.dram_tensor("e_out", [1, C * BC], F32, kind="ExternalOutput")
    d_v_out = nc.dram_tensor("v_out", [1, BC], F32, kind="ExternalOutput")

    with tile.TileContext(nc) as tc:
        with (
            tc.tile_pool(name="const", bufs=1) as cp,
            tc.tile_pool(name="work", bufs=2) as sb,
            tc.tile_pool(name="psum", bufs=1, space="PSUM") as pp,
        ):
            # ---- persistent tiles
            st_b = cp.tile([OBS + 1, BC], BF16)
            st_r = cp.tile([OBS + 1, BC], F32R)
            st_f = cp.tile([OBS + 1, BC], F32)
            we1s_b = cp.tile([OBS + 1, HID], BF16)
            we1a_b = cp.tile([128, HID], BF16)
            we2_b = [cp.tile([128, HID], BF16, tag=f"we2b{k}", name=f"we2b{k}") for k in range(2)]
            we2rT_b = [cp.tile([128, HID], BF16, tag=f"we2rTb{k}", name=f"we2rTb{k}") for k in range(2)]
            waTs_b = cp.tile([128, NSTEP * 2 * ACTD], BF16)
            we1s_f = cp.tile([OBS + 1, HID], F32R)
            we1a_f = cp.tile([128, HID], F32R)
            we2_f = [cp.tile([128, HID], F32R, tag=f"we2f{k}", name=f"we2f{k}") for k in range(2)]
            we3_f = cp.tile([128, 2], F32R)
            wc1_f = cp.tile([OBS + 1, HID], F32)
            wc2_f = [cp.tile([128, HID], F32, tag=f"wc2f{k}", name=f"wc2f{k}") for k in range(2)]
            wc3_f = [cp.tile([128, HID], F32, tag=f"wc3f{k}", name=f"wc3f{k}") for k in range(2)]
            wc4_f = cp.tile([128, 2], F32)
            be2 = cp.tile([128, 2], F32)
            bc2 = cp.tile([128, 2], F32)
            bc3 = cp.tile([128, 2], F32)
            aq = [cp.tile([128, BC], F32, tag=f"aq{q}", name=f"aq{q}") for q in range(NQ)]
            ab = [cp.tile([128, BC], BF16, tag=f"ab{q}", name=f"ab{q}") for q in range(NQ)]
            uq = [cp.tile([128, BC], F32, tag=f"uq{q}", name=f"uq{q}") for q in range(NQ)]
            af = [cp.tile([128, BC], F32R, tag=f"af{q}", name=f"af{q}") for q in range(NQ)]
            e_sb = cp.tile([1, C * BC], F32)
            v_sb = cp.tile([1, BC], F32)

            # ---- loads
            dma = nc.sync.dma_start
            dma(st_b[:], d_st_b[:])
            dma(st_r[:], d_st_r[:])
            dma(st_f[:], d_st_f[:])
            dma(we1s_b[:], d_we1s_b[:])
            dma(we1a_b[:], d_we1a_b[:])
            for k in range(2):
                dma(we2_b[k][:], d_we2_b[k * 128:(k + 1) * 128, :])
                dma(we2rT_b[k][:], d_we2rT_b[k * 128:(k + 1) * 128, :])
                dma(we2_f[k][:], d_we2_f[k * 128:(k + 1) * 128, :])
                dma(wc2_f[k][:], d_wc2_f[k * 128:(k + 1) * 128, :])
                dma(wc3_f[k][:], d_wc3_f[k * 128:(k + 1) * 128, :])
            dma(waTs_b[:], d_waTs_b[:])
            dma(we1s_f[:], d_we1s_f[:])
            dma(we1a_f[:], d_we1a_f[:])
            dma(we3_f[:], d_we3_f[:])
            dma(wc1_f[:], d_wc1_f[:])
            dma(wc4_f[:], d_wc4_f[:])
            dma(be2[:], d_be2[:])
            dma(bc2[:], d_bc2[:])
            dma(bc3[:], d_bc3[:])
            for q in range(NQ):
                dma(aq[q][:], d_aq0[q])
                nc.vector.memset(uq[q][:], 0.0)
                nc.vector.tensor_copy(ab[q][:], aq[q][:])

            mm = nc.tensor.matmul
            act = nc.scalar.activation
            stt = nc.vector.scalar_tensor_tensor
            ts = nc.vector.tensor_scalar
            tt = nc.vector.tensor_tensor

            # ================= MCMC loop =================
            for t in range(NSTEP):
                s2 = float(steps[t]) / 2.0
                offt = []
                for q in range(NQ):
                    o = sb.tile([128, BC], F32, tag="off", bufs=3)
                    dma(o[:], d_offq[t, q])
                    offt.append(o)
                gaq = None
                for c in range(NCHUNK):
                    q, j = divmod(c, 4)
                    a_sl = ab[q][32 * j:32 * j + 8, :]
                    h1 = sb.tile([128, 2 * BC], BF16, tag="h1", bufs=2)
                    for h in range(2):
                        hs = slice(h * 128, (h + 1) * 128)
                        os_ = slice(h * BC, (h + 1) * BC)
                        pre1 = pp.tile([128, BC], F32, tag="pre", bufs=4, name="pre1")
                        mm(pre1[:], we1s_b[:, hs], st_b[:], start=True, stop=False)
                        mm(pre1[:], we1a_b[32 * j:32 * j + 8, hs], a_sl,
                           start=False, stop=True, tile_position=(32 * j, 0))
                        act(h1[:, os_], pre1[:], AFT.Tanh)
                    h2 = sb.tile([128, 2 * BC], BF16, tag="h2", bufs=2)
                    for h in range(2):
                        hs = slice(h * 128, (h + 1) * 128)
                        os_ = slice(h * BC, (h + 1) * BC)
                        pre2 = pp.tile([128, BC], F32, tag="pre", bufs=4, name="pre2")
                        mm(pre2[:], we2_b[0][:, hs], h1[:, 0:BC], start=True, stop=False)
                        mm(pre2[:], we2_b[1][:, hs], h1[:, BC:2 * BC], start=False, stop=True)
                        act(h2[:, os_], pre2[:], AFT.Tanh, bias=be2[:, h:h + 1])
                    sq2 = sb.tile([128, 2 * BC], BF16, tag="sq", bufs=3)
                    tt(sq2[:], h2[:], h2[:], op=ALU.mult)
                    d2 = sb.tile([128, 2 * BC], BF16, tag="d", bufs=3)
                    ts(d2[:], sq2[:], -1.0, 1.0, op0=ALU.mult, op1=ALU.add)
                    sq1 = sb.tile([128, 2 * BC], BF16, tag="sq", bufs=3)
                    tt(sq1[:], h1[:], h1[:], op=ALU.mult)
                    d1 = sb.tile([128, 2 * BC], BF16, tag="d", bufs=3)
                    ts(d1[:], sq1[:], -1.0, 1.0, op0=ALU.mult, op1=ALU.add)
                    g1 = sb.tile([128, 2 * BC], BF16, tag="g1", bufs=2)
                    for h in range(2):
                        hs = slice(h * 128, (h + 1) * 128)
                        os_ = slice(h * BC, (h + 1) * BC)
                        dh1 = pp.tile([128, BC], F32, tag="dh1", bufs=2, name="dh1")
                        mm(dh1[:], we2rT_b[0][:, hs], d2[:, 0:BC], start=True, stop=False)
                        mm(dh1[:], we2rT_b[1][:, hs], d2[:, BC:2 * BC], start=False, stop=True)
                        tt(g1[:, os_], dh1[:], d1[:, os_], op=ALU.mult)
                    if j == 0:
                        gaq = pp.tile([128, BC], F32, tag="ga", bufs=2)
                    for k in range(2):
                        ws = slice((t * 2 + k) * ACTD, (t * 2 + k + 1) * ACTD)
                        mm(gaq[32 * j:32 * j + 8, :], waTs_b[:, ws], g1[:, k * BC:(k + 1) * BC],
                           start=(k == 0), stop=(k == 1), tile_position=(0, 32 * j))
                    if j == 3 or c == NCHUNK - 1:
                        # quad complete: clip + update + bf16 refresh
                        ts(uq[q][:], gaq[:], s2, -s2, op0=ALU.min, op1=ALU.max)
                        x = sb.tile([128, BC], F32, tag="x", bufs=2)
                        stt(x[:], offt[q][:], -1.0, uq[q][:], op0=ALU.mult, op1=ALU.add)
                        tt(aq[q][:], aq[q][:], x[:], op=ALU.add)
                        ts(aq[q][:], aq[q][:], 1.0, -1.0, op0=ALU.min, op1=ALU.max)
                        nc.scalar.copy(ab[q][:], aq[q][:])

            # ================= final energy pass (fp32r) =================
            for q in range(NQ):
                nc.vector.tensor_copy(af[q][:], aq[q][:])
            for c in range(NCHUNK):
                q, j = divmod(c, 4)
                a_sl = af[q][32 * j:32 * j + 8, :]
                h1f = sb.tile([128, 2 * BC], F32R, tag="h1f", bufs=2)
                for h in range(2):
                    hs = slice(h * 128, (h + 1) * 128)
                    os_ = slice(h * BC, (h + 1) * BC)
                    pre1 = pp.tile([128, BC], F32, tag="pre", bufs=4, name="pre1f")
                    mm(pre1[:], we1s_f[:, hs], st_r[:], start=True, stop=False)
                    mm(pre1[:], we1a_f[32 * j:32 * j + 8, hs], a_sl,
                       start=False, stop=True, tile_position=(32 * j, 0))
                    act(h1f[:, os_], pre1[:], AFT.Tanh)
                h2f = sb.tile([128, 2 * BC], F32R, tag="h2f", bufs=2)
                for h in range(2):
                    hs = slice(h * 128, (h + 1) * 128)
                    os_ = slice(h * BC, (h + 1) * BC)
                    pre2 = pp.tile([128, BC], F32, tag="pre", bufs=4, name="pre2f")
                    mm(pre2[:], we2_f[0][:, hs], h1f[:, 0:BC], start=True, stop=False)
                    mm(pre2[:], we2_f[1][:, hs], h1f[:, BC:2 * BC], start=False, stop=True)
                    act(h2f[:, os_], pre2[:], AFT.Tanh, bias=be2[:, h:h + 1])
                ev = pp.tile([1, BC], F32, tag="ga", bufs=2)
                for k in range(2):
                    mm(ev[:], we3_f[:, k:k + 1], h2f[:, k * BC:(k + 1) * BC],
                       start=(k == 0), stop=(k == 1))
                nc.scalar.copy(e_sb[0:1, c * BC:(c + 1) * BC], ev[:])

            # ================= critic (fp32, bc1 folded into wc1 row 64) ====
            t1 = sb.tile([128, 2 * BC], F32, tag="ct1", bufs=1)
            for h in range(2):
                hs = slice(h * 128, (h + 1) * 128)
                os_ = slice(h * BC, (h + 1) * BC)
                cpre = pp.tile([128, BC], F32, tag="pre", bufs=4, name="cpre")
                mm(cpre[:], wc1_f[:, hs], st_f[:], start=True, stop=True)
                act(t1[:, os_], cpre[:], AFT.Tanh)
            t2 = sb.tile([128, 2 * BC], F32, tag="ct2", bufs=1)
            for h in range(2):
                hs = slice(h * 128, (h + 1) * 128)
                os_ = slice(h * BC, (h + 1) * BC)
                cpre2 = pp.tile([128, BC], F32, tag="pre", bufs=4, name="cpre2")
                mm(cpre2[:], wc2_f[0][:, hs], t1[:, 0:BC], start=True, stop=False)
                mm(cpre2[:], wc2_f[1][:, hs], t1[:, BC:2 * BC], start=False, stop=True)
                act(t2[:, os_], cpre2[:], AFT.Tanh, bias=bc2[:, h:h + 1])
            t3 = sb.tile([128, 2 * BC], F32, tag="ct3", bufs=1)
            for h in range(2):
                hs = slice(h * 128, (h + 1) * 128)
                os_ = slice(h * BC, (h + 1) * BC)
                cpre3 = pp.tile([128, BC], F32, tag="dh1", bufs=2, name="cpre3")
                mm(cpre3[:], wc3_f[0][:, hs], t2[:, 0:BC], start=True, stop=False)
                mm(cpre3[:], wc3_f[1][:, hs], t2[:, BC:2 * BC], start=False, stop=True)
                act(t3[:, os_], cpre3[:], AFT.Tanh, bias=bc3[:, h:h + 1])
            vv = pp.tile([1, BC], F32, tag="ga", bufs=2)
            for k in range(2):
                mm(vv[:], wc4_f[:, k:k + 1], t3[:, k * BC:(k + 1) * BC],
                   start=(k == 0), stop=(k == 1))
            nc.scalar.copy(v_sb[:], vv[:])

            # ================= outputs =================
            dma(d_a_out[:], aq[0][0:8, :])
            dma(d_e_out[:], e_sb[:])
            dma(d_v_out[:], v_sb[:])

    nc.finalize()
    _CACHE["nc"] = nc
    return nc


# ---------------------------------------------------------------- host side

def _host_noise(temperature):
    import jax
    import jax.numpy as jnp
    cpu = jax.devices("cpu")[0]
    with jax.default_device(cpu):
        T = np.float32(np.exp(np.float32(temperature[0])))
        key = jax.random.key(42)
        k_init, k_loop = jax.random.split(key)
        a0 = np.asarray(jnp.tanh(jax.random.normal(k_init, (C, B, ACTD))))
        noises = []
        k = k_loop
        for _ in range(NSTEP):
            k, kn = jax.random.split(k)
            noises.append(np.asarray(jax.random.normal(kn, (C, B, ACTD))))
    steps = _stepsizes()
    off = [np.float32(steps[t] * T) * noises[t] for t in range(NSTEP)]
    return a0, off, steps, T


def _to_quads(arr):
    """arr [C, BCrows, ACTD] -> [NQ, 128, BC] (chunk j of quad at partitions 32j..32j+8)."""
    out = np.zeros((NQ, 128, BC), np.float32)
    for i in range(C):
        q, j = divmod(i, 4)
        out[q, 32 * j:32 * j + 8, :] = arr[i].T
    return out


def _prep_in_maps(inputs):
    f32 = lambda x: np.ascontiguousarray(np.asarray(x, np.float32))
    b16 = lambda x: np.ascontiguousarray(np.asarray(x, np.float32).astype(ml_dtypes.bfloat16))
    states = f32(inputs["states"])
    We1 = f32(inputs["We1"]); be1 = f32(inputs["be1"])
    We2 = f32(inputs["We2"]); be2 = f32(inputs["be2"])
    We3 = f32(inputs["We3"]); be3 = f32(inputs["be3"])
    Wc1 = f32(inputs["Wc1"]); bc1 = f32(inputs["bc1"])
    Wc2 = f32(inputs["Wc2"]); bc2 = f32(inputs["bc2"])
    Wc3 = f32(inputs["Wc3"]); bc3 = f32(inputs["bc3"])
    Wc4 = f32(inputs["Wc4"]); bc4 = f32(inputs["bc4"])
    temperature = f32(inputs["temperature"])

    a0, off, steps, T = _host_noise(temperature)

    we3v = We3[:, 0]
    we2rT = we3v[:, None] * We2.T                       # [256,256] lhsT for DH1
    waTs = np.zeros((128, NSTEP * 2 * ACTD), np.float32)
    for t in range(NSTEP):
        w = np.float32(-(steps[t] / np.float32(2.0))) * We1[OBS:].T    # [256, 8]
        for k in range(2):
            waTs[:, (t * 2 + k) * ACTD:(t * 2 + k + 1) * ACTD] = w[k * 128:(k + 1) * 128]

    we1a4 = np.zeros((128, HID), np.float32)
    for j in range(4):
        we1a4[32 * j:32 * j + ACTD] = We1[OBS:]

    we1sb = np.concatenate([We1[:OBS], be1[None, :]], axis=0)      # [65, 256]
    wc1b = np.concatenate([Wc1, bc1[None, :]], axis=0)             # [65, 256]

    pack2 = lambda v: np.ascontiguousarray(np.stack([v[:128], v[128:]], axis=1))  # [256] -> [128, 2]
    shared = {
        "we1s_b": b16(we1sb), "we1a_b": b16(we1a4),
        "we2_b": b16(We2), "we2rT_b": b16(we2rT), "waTs_b": b16(waTs),
        "we1s_f": we1sb, "we1a_f": we1a4, "we2_f": We2,
        "we3_f": pack2(We3[:, 0]), "wc1_f": wc1b, "wc2_f": Wc2, "wc3_f": Wc3,
        "wc4_f": pack2(Wc4[:, 0]),
        "be2": pack2(be2), "bc2": pack2(bc2), "bc3": pack2(bc3),
    }

    in_maps = []
    for cc in range(NCORE):
        bs = slice(cc * BC, (cc + 1) * BC)
        st = np.ascontiguousarray(
            np.concatenate([states[bs].T, np.ones((1, BC), np.float32)], axis=0))  # [65, 512]
        aq0 = _to_quads(a0[:, bs, :])
        offq = np.ascontiguousarray(
            np.stack([_to_quads(off[t][:, bs, :]) for t in range(NSTEP)]))
        m = dict(shared)
        m.update({"st_b": b16(st), "st_r": st, "st_f": st, "aq0": aq0, "offq": offq})
        in_maps.append(m)

    meta = {"T": T, "be3": np.float32(be3[0]), "bc4": np.float32(bc4[0])}
    return in_maps, meta


def _postprocess(results, meta):
    T = meta["T"]
    actions = np.zeros((B, ACTD), np.float32)
    E = np.zeros((C, B), np.float32)
    V = np.zeros((B, 1), np.float32)
    for cc, r in enumerate(results):
        bs = slice(cc * BC, (cc + 1) * BC)
        actions[bs] = r["a_out"].T
        E[:, bs] = r["e_out"].reshape(C, BC)
        V[bs, 0] = r["v_out"][0]
    E += meta["be3"]
    V += meta["bc4"]
    energy = E[0]
    neg = -E[1:] / T
    m = neg.max(axis=0)
    lse = m + np.log(np.exp(neg - m).sum(axis=0))
    log_prob = (-energy / T - lse).astype(np.float32)
    entropy = np.float32(T * log_prob.mean(dtype=np.float64))
    return actions, log_prob, entropy, V


def _run(inputs, **spmd_kwargs):
    nc = _build()
    in_maps, meta = _prep_in_maps(inputs)
    res = bass_utils.run_bass_kernel_spmd(nc, in_maps, core_ids=list(range(NCORE)), **spmd_kwargs)
    return _postprocess(res.results, meta), res


def kernel(**inputs):
    out, _ = _run(inputs)
    return out
